# revision 1
# baseline (speedup 1.0000x reference)
"""Trainium2 Bass kernel for the additive-attention module:
    q = query @ Wq.T + bq                                  (N, H)
    r = einsum("nlh,oh->nol", ref, Wr) + br[None,:,None]   (N, O, L)
    logits = 10 * tanh(einsum("h,nhl->nl", v, tanh(q[:,:,None] + r)))
Returns (r, logits).

Data-parallel over batch N across 8 NeuronCores (16 rows each); params
replicated. Per core: PE transposes ref tiles to put H on partitions,
fp32r matmuls against the pre-transposed Wr, ACT applies tanh with the
per-(n,o) bias, a second PE reduction dots with v.
"""
import sys

sys.path.insert(0, "/opt/trn_rl_repo")

import numpy as np
import concourse.bass as bass
import concourse.tile as tile
from concourse import mybir
from concourse.vector_clock import ScopedClock

F32 = mybir.dt.float32
F32R = mybir.dt.float32r
TANH = mybir.ActivationFunctionType.Tanh
IDENT = mybir.ActivationFunctionType.Identity

N, L, H = 128, 2048, 512
CLIPPING = 10.0
NCORES = 8
NLOC = N // NCORES          # 16 batch rows per core
P = 128                     # partitions
HC = H // P                 # 4 contraction chunks
OC = H // P                 # 4 output-row chunks
LC = L // 512               # 4 l-chunks of 512
LS = 512 // P               # 4 l-subtiles per chunk

_ENGINE_HANDLES = {
    mybir.EngineType.PE: "tensor",
    mybir.EngineType.DVE: "vector",
    mybir.EngineType.Activation: "scalar",
    mybir.EngineType.Pool: "gpsimd",
    mybir.EngineType.SP: "sync",
}


def _split_sync_waits(nc, max_waits=1):
    """This walrus build rejects instructions carrying more than a couple of
    semaphore waits; move the excess onto same-engine NOPs placed right
    before the instruction (same program-order semantics)."""
    for bb in nc.main_func.blocks:
        new_insts = []
        for inst in bb.instructions:
            si = inst.sync_info
            if si is not None and si.on_wait is not None and len(si.on_wait) > max_waits:
                waits = list(si.on_wait)
                excess = waits[: len(waits) - max_waits]
                handle = getattr(nc, _ENGINE_HANDLES[inst.engine])
                for j in range(0, len(excess), max_waits):
                    nop = handle.nop(nofuse=True).ins
                    for b2 in nc.main_func.blocks:
                        if b2.instructions and b2.instructions[-1] is nop:
                            b2.instructions.pop()
                            break
                    nsi = nop.sync_info or mybir.SyncInfo(on_wait=[], on_update=[])
                    nsi.on_wait = excess[j : j + max_waits]
                    nop.sync_info = nsi
                    new_insts.append(nop)
                si.on_wait = waits[len(waits) - max_waits :]
                inst.sync_info = si
            new_insts.append(inst)
        bb.instructions = new_insts


class _TC(tile.TileContext):
    """TileContext whose tail drain splits its sem waits the same way."""

    MAX_WAITS = 1

    def _drain_and_barrier(self, tick_clock, wait_clock):
        drain_inst = self.nc.sync.drain()
        wait_clock.add_sem_waits(
            drain_inst.ins, ScopedClock({None: tick_clock.global_clock})
        )
        waits = list(drain_inst.ins.sync_info.on_wait)
        if len(waits) > self.MAX_WAITS:
            si = drain_inst.ins.sync_info
            si.on_wait = waits[: self.MAX_WAITS]
            drain_inst.ins.sync_info = si
            for i in range(self.MAX_WAITS, len(waits), self.MAX_WAITS):
                extra = self.nc.sync.drain()
                esi = extra.ins.sync_info or mybir.SyncInfo(on_wait=[], on_update=[])
                esi.on_wait = waits[i : i + self.MAX_WAITS]
                extra.ins.sync_info = esi
        self.nc.all_engine_barrier()
        assert self.sems is not None
        popped = self.nc._tile_sem_poison_stack.pop()
        assert popped is self._sem_poison
        self.nc.clear_and_free_semaphores(list(self.sems.allocated().values()))
        self.nc.all_engine_barrier()


def build_kernel():
    nc = bass.Bass(trn_type="TRN2")

    query = nc.dram_tensor("query", [NLOC, H], F32, kind="ExternalInput")
    ref = nc.dram_tensor("ref", [NLOC, L, H], F32, kind="ExternalInput")
    wq = nc.dram_tensor("wq", [H, H], F32, kind="ExternalInput")
    bq = nc.dram_tensor("bq", [1, H], F32, kind="ExternalInput")
    wr = nc.dram_tensor("wr", [H, H], F32, kind="ExternalInput")
    br = nc.dram_tensor("br", [1, H], F32, kind="ExternalInput")
    v = nc.dram_tensor("v", [1, H], F32, kind="ExternalInput")
    ident = nc.dram_tensor("ident", [P, P], F32, kind="ExternalInput")

    r_out = nc.dram_tensor("r_out", [NLOC, H, L], F32, kind="ExternalOutput")
    logits_out = nc.dram_tensor("logits_out", [NLOC, L], F32, kind="ExternalOutput")

    with _TC(nc) as tc:
        with (
            tc.tile_pool(name="const", bufs=1) as cpool,
            tc.tile_pool(name="ref_nat", bufs=3) as rn_pool,
            tc.tile_pool(name="ref_t", bufs=2) as rt_pool,
            tc.tile_pool(name="outs", bufs=4) as out_pool,
            tc.tile_pool(name="mm_psum", bufs=2, space="PSUM") as mm_psum,
            tc.tile_pool(name="tp_psum", bufs=3, space="PSUM") as tp_psum,
            tc.tile_pool(name="log_psum", bufs=2, space="PSUM") as log_psum,
        ):
            # ---- constants / weights ----
            id_sb = cpool.tile([P, P], F32)
            nc.sync.dma_start(id_sb[:], ident[:])

            # Wr natural [o_sub, oc, h] then transpose -> wrt [h_sub, hc, o] (f32r)
            wr_nat = cpool.tile([P, OC, H], F32)
            nc.sync.dma_start(wr_nat[:], wr.rearrange("(oc p) h -> p oc h", p=P))
            wrt = cpool.tile([P, HC, H], F32R)
            for hc in range(HC):
                for oc in range(OC):
                    pst = tp_psum.tile([P, P], F32)
                    nc.tensor.transpose(
                        pst[:], wr_nat[:, oc, bass.ts(hc, P)], id_sb[:]
                    )
                    nc.vector.tensor_copy(wrt[:, hc, bass.ts(oc, P)], pst[:])

            # Wq natural then transpose -> wqt [h_sub, hc, o] (f32, exact matmul)
            wq_nat = cpool.tile([P, OC, H], F32)
            nc.sync.dma_start(wq_nat[:], wq.rearrange("(oc p) h -> p oc h", p=P))
            wqt = cpool.tile([P, HC, H], F32)
            for hc in range(HC):
                for oc in range(OC):
                    pst = tp_psum.tile([P, P], F32)
                    nc.tensor.transpose(
                        pst[:], wq_nat[:, oc, bass.ts(hc, P)], id_sb[:]
                    )
                    nc.vector.tensor_copy(wqt[:, hc, bass.ts(oc, P)], pst[:])

            # bias vectors as [o_sub, oc]
            bq_sb = cpool.tile([P, OC], F32)
            nc.sync.dma_start(bq_sb[:], bq.rearrange("one (oc p) -> p (one oc)", p=P))
            br_sb = cpool.tile([P, OC], F32)
            nc.sync.dma_start(br_sb[:], br.rearrange("one (oc p) -> p (one oc)", p=P))
            bqbr = cpool.tile([P, OC], F32)
            nc.vector.tensor_add(bqbr[:], bq_sb[:], br_sb[:])
            v_f32 = cpool.tile([P, OC], F32)
            nc.sync.dma_start(v_f32[:], v.rearrange("one (oc p) -> p (one oc)", p=P))
            v_sb = cpool.tile([P, OC], F32R)
            nc.vector.tensor_copy(v_sb[:], v_f32[:])

            # ---- q = query @ Wq.T + bq (+ br), laid out [o_sub, oc, n] ----
            q_nat = cpool.tile([NLOC, H], F32)
            nc.sync.dma_start(q_nat[:], query[:])
            qt = cpool.tile([P, HC, NLOC], F32)
            for hc in range(HC):
                pst = tp_psum.tile([P, P], F32)
                nc.tensor.transpose(
                    pst[:, :NLOC], q_nat[:, bass.ts(hc, P)], id_sb[:NLOC, :NLOC]
                )
                nc.vector.tensor_copy(qt[:, hc, :], pst[:, :NLOC])
            qb = cpool.tile([P, OC, NLOC], F32)
            for oc in range(OC):
                psq = mm_psum.tile([P, 512], F32, tag="mm")
                for hc in range(HC):
                    nc.tensor.matmul(
                        psq[:, :NLOC],
                        wqt[:, hc, bass.ts(oc, P)],
                        qt[:, hc, :],
                        start=(hc == 0),
                        stop=(hc == HC - 1),
                    )
                nc.scalar.activation(
                    qb[:, oc, :], psq[:, :NLOC], IDENT, bias=bqbr[:, oc : oc + 1]
                )

            # ---- main loop over (n, lc) ----
            for n in range(NLOC):
                logits_n = out_pool.tile([1, L], F32, tag="logits_n")
                for lc in range(LC):
                    # natural ref tile [l_sub(p), ls, h]
                    rn = rn_pool.tile([P, LS, H], F32, tag="rn")
                    nc.sync.dma_start(
                        rn[:],
                        ref[n].rearrange(
                            "(lc ls p) h -> lc p ls h", lc=LC, ls=LS, p=P
                        )[lc],
                    )
                    # transpose to [h_sub, hc, l] f32r
                    rt = rt_pool.tile([P, HC, 512], F32R, tag="rt")
                    for hc in range(HC):
                        for ls in range(LS):
                            pst = tp_psum.tile([P, P], F32)
                            nc.tensor.transpose(
                                pst[:], rn[:, ls, bass.ts(hc, P)], id_sb[:]
                            )
                            nc.vector.tensor_copy(rt[:, hc, bass.ts(ls, P)], pst[:])

                    pslog = log_psum.tile([1, 512], F32, tag="pslog")
                    for oc in range(OC):
                        psr = mm_psum.tile([P, 512], F32, tag="mm")
                        for hc in range(HC):
                            nc.tensor.matmul(
                                psr[:],
                                wrt[:, hc, bass.ts(oc, P)],
                                rt[:, hc, :],
                                start=(hc == 0),
                                stop=(hc == HC - 1),
                            )
                        # r slab: psum + br  -> HBM
                        r_sb = out_pool.tile([P, 512], F32, tag="r_sb")
                        if oc % 2 == 0:
                            nc.vector.tensor_scalar_add(
                                r_sb[:], psr[:], br_sb[:, oc : oc + 1]
                            )
                        else:
                            nc.scalar.activation(
                                r_sb[:], psr[:], IDENT, bias=br_sb[:, oc : oc + 1]
                            )
                        nc.sync.dma_start(
                            r_out[n, bass.ts(oc, P), bass.ts(lc, 512)], r_sb[:]
                        )
                        # tanh(q + r) -> f32r
                        th = out_pool.tile([P, 512], F32R, tag="th")
                        nc.scalar.activation(
                            th[:], psr[:], TANH, bias=qb[:, oc, n : n + 1]
                        )
                        # logits partial: v . tanh, contract over o_sub
                        nc.tensor.matmul(
                            pslog[:],
                            v_sb[:, oc : oc + 1],
                            th[:],
                            start=(oc == 0),
                            stop=(oc == OC - 1),
                        )
                    # logits tail: 10 * tanh(acc)
                    lg = out_pool.tile([1, 512], F32, tag="lg")
                    nc.scalar.activation(lg[:], pslog[:], TANH)
                    nc.vector.tensor_scalar_mul(
                        logits_n[:, bass.ts(lc, 512)], lg[:], CLIPPING
                    )
                nc.sync.dma_start(logits_out[n : n + 1, :], logits_n[:])

    _split_sync_waits(nc, max_waits=1)
    return nc


_NC_CACHE = None


def _get_nc():
    global _NC_CACHE
    if _NC_CACHE is None:
        _NC_CACHE = build_kernel()
    return _NC_CACHE


def kernel(query, ref, Wq, bq, Wr, br, v):
    from concourse.bass_utils import run_bass_kernel_spmd

    query = np.ascontiguousarray(query, dtype=np.float32)
    ref = np.ascontiguousarray(ref, dtype=np.float32)
    ident = np.eye(P, dtype=np.float32)
    shared = {
        "wq": np.ascontiguousarray(Wq, dtype=np.float32),
        "bq": np.ascontiguousarray(bq, dtype=np.float32).reshape(1, H),
        "wr": np.ascontiguousarray(Wr, dtype=np.float32),
        "br": np.ascontiguousarray(br, dtype=np.float32).reshape(1, H),
        "v": np.ascontiguousarray(v, dtype=np.float32).reshape(1, H),
        "ident": ident,
    }
    in_maps = []
    for c in range(NCORES):
        sl = slice(c * NLOC, (c + 1) * NLOC)
        in_maps.append({"query": query[sl], "ref": ref[sl], **shared})

    nc = _get_nc()
    res = run_bass_kernel_spmd(nc, in_maps, core_ids=list(range(NCORES)))
    r_full = np.concatenate([res.results[c]["r_out"] for c in range(NCORES)], axis=0)
    logits_full = np.concatenate(
        [res.results[c]["logits_out"] for c in range(NCORES)], axis=0
    )
    return (r_full, logits_full)


# revision 5
# speedup vs baseline: 1.1487x; 1.1487x over previous
"""Trainium2 Bass kernel for the additive-attention module:
    q = query @ Wq.T + bq                                  (N, H)
    r = einsum("nlh,oh->nol", ref, Wr) + br[None,:,None]   (N, O, L)
    logits = 10 * tanh(einsum("h,nhl->nl", v, tanh(q[:,:,None] + r)))
Returns (r, logits).

Data-parallel over batch N across 8 NeuronCores (16 rows each); params
replicated. Per core: PE transposes ref tiles to put H on partitions,
fp32r matmuls against the pre-transposed Wr, ACT applies tanh with the
per-(n,o) bias, a second PE reduction dots with v.
"""
import sys

sys.path.insert(0, "/opt/trn_rl_repo")

import numpy as np
import concourse.bass as bass
import concourse.tile as tile
from concourse import mybir
from concourse.vector_clock import ScopedClock

F32 = mybir.dt.float32
F32R = mybir.dt.float32r
TANH = mybir.ActivationFunctionType.Tanh
IDENT = mybir.ActivationFunctionType.Identity

N, L, H = 128, 2048, 512
CLIPPING = 10.0
NCORES = 8
NLOC = N // NCORES          # 16 batch rows per core
P = 128                     # partitions
HC = H // P                 # 4 contraction chunks
OC = H // P                 # 4 output-row chunks
LC = L // 512               # 4 l-chunks of 512
LS = 512 // P               # 4 l-subtiles per chunk

_ENGINE_HANDLES = {
    mybir.EngineType.PE: "tensor",
    mybir.EngineType.DVE: "vector",
    mybir.EngineType.Activation: "scalar",
    mybir.EngineType.Pool: "gpsimd",
    mybir.EngineType.SP: "sync",
}


def _split_sync_waits(nc, max_waits=1):
    """This walrus build rejects instructions carrying more than a couple of
    semaphore waits; move the excess onto same-engine NOPs placed right
    before the instruction (same program-order semantics)."""
    for bb in nc.main_func.blocks:
        new_insts = []
        for inst in bb.instructions:
            si = inst.sync_info
            if si is not None and si.on_wait is not None and len(si.on_wait) > max_waits:
                waits = list(si.on_wait)
                excess = waits[: len(waits) - max_waits]
                handle = getattr(nc, _ENGINE_HANDLES[inst.engine])
                for j in range(0, len(excess), max_waits):
                    nop = handle.nop(nofuse=True).ins
                    for b2 in nc.main_func.blocks:
                        if b2.instructions and b2.instructions[-1] is nop:
                            b2.instructions.pop()
                            break
                    nsi = nop.sync_info or mybir.SyncInfo(on_wait=[], on_update=[])
                    nsi.on_wait = excess[j : j + max_waits]
                    nop.sync_info = nsi
                    new_insts.append(nop)
                si.on_wait = waits[len(waits) - max_waits :]
                inst.sync_info = si
            new_insts.append(inst)
        bb.instructions = new_insts


class _TC(tile.TileContext):
    """TileContext whose tail drain splits its sem waits the same way."""

    MAX_WAITS = 1

    def _drain_and_barrier(self, tick_clock, wait_clock):
        drain_inst = self.nc.sync.drain()
        wait_clock.add_sem_waits(
            drain_inst.ins, ScopedClock({None: tick_clock.global_clock})
        )
        waits = list(drain_inst.ins.sync_info.on_wait)
        if len(waits) > self.MAX_WAITS:
            si = drain_inst.ins.sync_info
            si.on_wait = waits[: self.MAX_WAITS]
            drain_inst.ins.sync_info = si
            for i in range(self.MAX_WAITS, len(waits), self.MAX_WAITS):
                extra = self.nc.sync.drain()
                esi = extra.ins.sync_info or mybir.SyncInfo(on_wait=[], on_update=[])
                esi.on_wait = waits[i : i + self.MAX_WAITS]
                extra.ins.sync_info = esi
        self.nc.all_engine_barrier()
        assert self.sems is not None
        popped = self.nc._tile_sem_poison_stack.pop()
        assert popped is self._sem_poison
        self.nc.clear_and_free_semaphores(list(self.sems.allocated().values()))
        self.nc.all_engine_barrier()


def build_kernel():
    nc = bass.Bass(trn_type="TRN2")

    query = nc.dram_tensor("query", [NLOC, H], F32, kind="ExternalInput")
    ref = nc.dram_tensor("ref", [NLOC, L, H], F32, kind="ExternalInput")
    wq = nc.dram_tensor("wq", [H, H], F32, kind="ExternalInput")
    bq = nc.dram_tensor("bq", [1, H], F32, kind="ExternalInput")
    wr = nc.dram_tensor("wr", [H, H], F32, kind="ExternalInput")
    br = nc.dram_tensor("br", [1, H], F32, kind="ExternalInput")
    v = nc.dram_tensor("v", [1, H], F32, kind="ExternalInput")
    ident = nc.dram_tensor("ident", [P, P], F32, kind="ExternalInput")

    r_out = nc.dram_tensor("r_out", [NLOC, H, L], F32, kind="ExternalOutput")
    logits_out = nc.dram_tensor("logits_out", [NLOC, L], F32, kind="ExternalOutput")

    with _TC(nc) as tc:
        with (
            tc.tile_pool(name="const", bufs=1) as cpool,
            tc.tile_pool(name="ref_nat", bufs=3) as rn_pool,
            tc.tile_pool(name="ref_t", bufs=2) as rt_pool,
            tc.tile_pool(name="outs", bufs=6) as out_pool,
            tc.tile_pool(name="mm_psum", bufs=3, space="PSUM") as mm_psum,
            tc.tile_pool(name="tp_psum", bufs=3, space="PSUM") as tp_psum,
            tc.tile_pool(name="log_psum", bufs=2, space="PSUM") as log_psum,
        ):
            # ---- constants / weights ----
            id_sb = cpool.tile([P, P], F32)
            nc.sync.dma_start(id_sb[:], ident[:])

            # Wr natural [o_sub, oc, h] then transpose -> wrt [h_sub, hc, o] (f32r)
            wr_nat = cpool.tile([P, OC, H], F32)
            nc.sync.dma_start(wr_nat[:], wr.rearrange("(oc p) h -> p oc h", p=P))
            wrt = cpool.tile([P, HC, H], F32R)
            for hc in range(HC):
                pst = tp_psum.tile([P, 512], F32, tag="pst")
                for oc in range(OC):
                    nc.tensor.transpose(
                        pst[:, bass.ts(oc, P)], wr_nat[:, oc, bass.ts(hc, P)], id_sb[:]
                    )
                nc.vector.tensor_copy(wrt[:, hc, :], pst[:])

            # Wq natural then transpose -> wqt [h_sub, hc, o] (f32, exact matmul)
            wq_nat = cpool.tile([P, OC, H], F32)
            nc.sync.dma_start(wq_nat[:], wq.rearrange("(oc p) h -> p oc h", p=P))
            wqt = cpool.tile([P, HC, H], F32)
            for hc in range(HC):
                pst = tp_psum.tile([P, 512], F32, tag="pst")
                for oc in range(OC):
                    nc.tensor.transpose(
                        pst[:, bass.ts(oc, P)], wq_nat[:, oc, bass.ts(hc, P)], id_sb[:]
                    )
                nc.vector.tensor_copy(wqt[:, hc, :], pst[:])

            # bias vectors as [o_sub, oc]
            bq_sb = cpool.tile([P, OC], F32)
            nc.sync.dma_start(bq_sb[:], bq.rearrange("one (oc p) -> p (one oc)", p=P))
            br_sb = cpool.tile([P, OC], F32)
            nc.sync.dma_start(br_sb[:], br.rearrange("one (oc p) -> p (one oc)", p=P))
            bqbr = cpool.tile([P, OC], F32)
            nc.vector.tensor_add(bqbr[:], bq_sb[:], br_sb[:])
            v_f32 = cpool.tile([P, OC], F32)
            nc.sync.dma_start(v_f32[:], v.rearrange("one (oc p) -> p (one oc)", p=P))
            v_sb = cpool.tile([P, OC], F32R)
            nc.vector.tensor_copy(v_sb[:], v_f32[:])

            # ---- q = query @ Wq.T + bq (+ br), laid out [o_sub, oc, n] ----
            q_nat = cpool.tile([NLOC, H], F32)
            nc.sync.dma_start(q_nat[:], query[:])
            qt = cpool.tile([P, HC, NLOC], F32)
            pstq = tp_psum.tile([P, 512], F32, tag="pst")
            for hc in range(HC):
                nc.tensor.transpose(
                    pstq[:, hc * NLOC : (hc + 1) * NLOC],
                    q_nat[:, bass.ts(hc, P)],
                    id_sb[:NLOC, :NLOC],
                )
            nc.vector.tensor_copy(qt[:], pstq[:, : HC * NLOC])
            qb = cpool.tile([P, OC, NLOC], F32)
            for oc in range(OC):
                psq = mm_psum.tile([P, 512], F32, tag="mm")
                for hc in range(HC):
                    nc.tensor.matmul(
                        psq[:, :NLOC],
                        wqt[:, hc, bass.ts(oc, P)],
                        qt[:, hc, :],
                        start=(hc == 0),
                        stop=(hc == HC - 1),
                    )
                nc.scalar.activation(
                    qb[:, oc, :], psq[:, :NLOC], IDENT, bias=bqbr[:, oc : oc + 1]
                )

            # ---- main loop over (n, lc) ----
            for n in range(NLOC):
                logits_n = out_pool.tile([1, L], F32, tag="logits_n")
                for lc in range(LC):
                    # natural ref tile [l_sub(p), ls, h]
                    rn = rn_pool.tile([P, LS, H], F32, tag="rn")
                    nc.gpsimd.dma_start(
                        rn[:],
                        ref[n].rearrange(
                            "(lc ls p) h -> lc p ls h", lc=LC, ls=LS, p=P
                        )[lc],
                    )
                    # transpose to [h_sub, hc, l] f32r: 4 transposes into one
                    # psum bank, then one wide rounding copy
                    rt = rt_pool.tile([P, HC, 512], F32R, tag="rt")
                    for hc in range(HC):
                        pst = tp_psum.tile([P, 512], F32, tag="pst")
                        for ls in range(LS):
                            nc.tensor.transpose(
                                pst[:, bass.ts(ls, P)], rn[:, ls, bass.ts(hc, P)], id_sb[:]
                            )
                        nc.vector.tensor_copy(rt[:, hc, :], pst[:])

                    pslog = log_psum.tile([1, 512], F32, tag="pslog")
                    for oc in range(OC):
                        psr = mm_psum.tile([P, 512], F32, tag="mm")
                        for hc in range(HC):
                            nc.tensor.matmul(
                                psr[:],
                                wrt[:, hc, bass.ts(oc, P)],
                                rt[:, hc, :],
                                start=(hc == 0),
                                stop=(hc == HC - 1),
                            )
                        # r slab: psum + br  -> HBM
                        r_sb = out_pool.tile([P, 512], F32, tag="r_sb")
                        if oc % 2 == 0:
                            nc.vector.tensor_scalar_add(
                                r_sb[:], psr[:], br_sb[:, oc : oc + 1]
                            )
                        else:
                            nc.scalar.activation(
                                r_sb[:], psr[:], IDENT, bias=br_sb[:, oc : oc + 1]
                            )
                        nc.sync.dma_start(
                            r_out[n, bass.ts(oc, P), bass.ts(lc, 512)], r_sb[:]
                        )
                        # tanh(q + r) -> f32r
                        th = out_pool.tile([P, 512], F32R, tag="th")
                        nc.scalar.activation(
                            th[:], psr[:], TANH, bias=qb[:, oc, n : n + 1]
                        )
                        # logits partial: v . tanh, contract over o_sub
                        nc.tensor.matmul(
                            pslog[:],
                            v_sb[:, oc : oc + 1],
                            th[:],
                            start=(oc == 0),
                            stop=(oc == OC - 1),
                        )
                    # logits tail: 10 * tanh(acc)
                    lg = out_pool.tile([1, 512], F32, tag="lg")
                    nc.scalar.activation(lg[:], pslog[:], TANH)
                    nc.vector.tensor_scalar_mul(
                        logits_n[:, bass.ts(lc, 512)], lg[:], CLIPPING
                    )
                nc.sync.dma_start(logits_out[n : n + 1, :], logits_n[:])

    _split_sync_waits(nc, max_waits=1)
    return nc


_NC_CACHE = None


def _get_nc():
    global _NC_CACHE
    if _NC_CACHE is None:
        _NC_CACHE = build_kernel()
    return _NC_CACHE


def kernel(query, ref, Wq, bq, Wr, br, v):
    from concourse.bass_utils import run_bass_kernel_spmd

    query = np.ascontiguousarray(query, dtype=np.float32)
    ref = np.ascontiguousarray(ref, dtype=np.float32)
    ident = np.eye(P, dtype=np.float32)
    shared = {
        "wq": np.ascontiguousarray(Wq, dtype=np.float32),
        "bq": np.ascontiguousarray(bq, dtype=np.float32).reshape(1, H),
        "wr": np.ascontiguousarray(Wr, dtype=np.float32),
        "br": np.ascontiguousarray(br, dtype=np.float32).reshape(1, H),
        "v": np.ascontiguousarray(v, dtype=np.float32).reshape(1, H),
        "ident": ident,
    }
    in_maps = []
    for c in range(NCORES):
        sl = slice(c * NLOC, (c + 1) * NLOC)
        in_maps.append({"query": query[sl], "ref": ref[sl], **shared})

    nc = _get_nc()
    res = run_bass_kernel_spmd(nc, in_maps, core_ids=list(range(NCORES)))
    r_full = np.concatenate([res.results[c]["r_out"] for c in range(NCORES)], axis=0)
    logits_full = np.concatenate(
        [res.results[c]["logits_out"] for c in range(NCORES)], axis=0
    )
    return (r_full, logits_full)


# revision 12
# speedup vs baseline: 1.1929x; 1.0385x over previous
"""Trainium2 Bass kernel for the additive-attention module:
    q = query @ Wq.T + bq                                  (N, H)
    r = einsum("nlh,oh->nol", ref, Wr) + br[None,:,None]   (N, O, L)
    logits = 10 * tanh(einsum("h,nhl->nl", v, tanh(q[:,:,None] + r)))
Returns (r, logits).

Data-parallel over batch N across 8 NeuronCores (16 rows each); params
replicated. Per core: PE transposes ref tiles to put H on partitions,
fp32r matmuls against the pre-transposed Wr, ACT applies tanh with the
per-(n,o) bias, a second PE reduction dots with v.
"""
import sys

sys.path.insert(0, "/opt/trn_rl_repo")

import numpy as np
import concourse.bass as bass
import concourse.tile as tile
from concourse import mybir
from concourse.vector_clock import ScopedClock

F32 = mybir.dt.float32
F32R = mybir.dt.float32r
TANH = mybir.ActivationFunctionType.Tanh
IDENT = mybir.ActivationFunctionType.Identity

N, L, H = 128, 2048, 512
CLIPPING = 10.0
NCORES = 8
NLOC = N // NCORES          # 16 batch rows per core
P = 128                     # partitions
HC = H // P                 # 4 contraction chunks
OC = H // P                 # 4 output-row chunks
LC = L // 512               # 4 l-chunks of 512
LS = 512 // P               # 4 l-subtiles per chunk

_ENGINE_HANDLES = {
    mybir.EngineType.PE: "tensor",
    mybir.EngineType.DVE: "vector",
    mybir.EngineType.Activation: "scalar",
    mybir.EngineType.Pool: "gpsimd",
    mybir.EngineType.SP: "sync",
}


def _split_sync_waits(nc, max_waits=1):
    """This walrus build rejects instructions carrying more than a couple of
    semaphore waits; move the excess onto same-engine NOPs placed right
    before the instruction (same program-order semantics)."""
    for bb in nc.main_func.blocks:
        new_insts = []
        for inst in bb.instructions:
            si = inst.sync_info
            if si is not None and si.on_wait is not None and len(si.on_wait) > max_waits:
                waits = list(si.on_wait)
                excess = waits[: len(waits) - max_waits]
                handle = getattr(nc, _ENGINE_HANDLES[inst.engine])
                for j in range(0, len(excess), max_waits):
                    nop = handle.nop(nofuse=True).ins
                    for b2 in nc.main_func.blocks:
                        if b2.instructions and b2.instructions[-1] is nop:
                            b2.instructions.pop()
                            break
                    nsi = nop.sync_info or mybir.SyncInfo(on_wait=[], on_update=[])
                    nsi.on_wait = excess[j : j + max_waits]
                    nop.sync_info = nsi
                    new_insts.append(nop)
                si.on_wait = waits[len(waits) - max_waits :]
                inst.sync_info = si
            new_insts.append(inst)
        bb.instructions = new_insts


class _TC(tile.TileContext):
    """TileContext whose tail drain splits its sem waits the same way."""

    MAX_WAITS = 1

    def _drain_and_barrier(self, tick_clock, wait_clock):
        drain_inst = self.nc.sync.drain()
        wait_clock.add_sem_waits(
            drain_inst.ins, ScopedClock({None: tick_clock.global_clock})
        )
        waits = list(drain_inst.ins.sync_info.on_wait)
        if len(waits) > self.MAX_WAITS:
            si = drain_inst.ins.sync_info
            si.on_wait = waits[: self.MAX_WAITS]
            drain_inst.ins.sync_info = si
            for i in range(self.MAX_WAITS, len(waits), self.MAX_WAITS):
                extra = self.nc.sync.drain()
                esi = extra.ins.sync_info or mybir.SyncInfo(on_wait=[], on_update=[])
                esi.on_wait = waits[i : i + self.MAX_WAITS]
                extra.ins.sync_info = esi
        self.nc.all_engine_barrier()
        assert self.sems is not None
        popped = self.nc._tile_sem_poison_stack.pop()
        assert popped is self._sem_poison
        self.nc.clear_and_free_semaphores(list(self.sems.allocated().values()))
        self.nc.all_engine_barrier()


def build_kernel():
    nc = bass.Bass(trn_type="TRN2")

    query = nc.dram_tensor("query", [NLOC, H], F32, kind="ExternalInput")
    ref = nc.dram_tensor("ref", [NLOC, L, H], F32, kind="ExternalInput")
    wq = nc.dram_tensor("wq", [H, H], F32, kind="ExternalInput")
    bq = nc.dram_tensor("bq", [1, H], F32, kind="ExternalInput")
    wr = nc.dram_tensor("wr", [H, H], F32, kind="ExternalInput")
    br = nc.dram_tensor("br", [1, H], F32, kind="ExternalInput")
    v = nc.dram_tensor("v", [1, H], F32, kind="ExternalInput")
    ident = nc.dram_tensor("ident", [P, P], F32, kind="ExternalInput")

    r_out = nc.dram_tensor("r_out", [NLOC, H, L], F32, kind="ExternalOutput")
    logits_out = nc.dram_tensor("logits_out", [NLOC, L], F32, kind="ExternalOutput")

    with _TC(nc) as tc:
        with (
            tc.tile_pool(name="const", bufs=1) as cpool,
            tc.tile_pool(name="ref_nat", bufs=3) as rn_pool,
            tc.tile_pool(name="ref_t", bufs=2) as rt_pool,
            tc.tile_pool(name="outs", bufs=6) as out_pool,
            tc.tile_pool(name="slabs", bufs=2) as slab_pool,
            tc.tile_pool(name="logits_sb", bufs=2) as log_pool,
            tc.tile_pool(name="mm_psum", bufs=3, space="PSUM") as mm_psum,
            tc.tile_pool(name="tp_psum", bufs=3, space="PSUM") as tp_psum,
            tc.tile_pool(name="log_psum", bufs=2, space="PSUM") as log_psum,
        ):
            # ---- constants / weights ----
            id_sb = cpool.tile([P, P], F32)
            nc.sync.dma_start(id_sb[:], ident[:])
            id_r = cpool.tile([P, P], F32R)
            nc.vector.tensor_copy(id_r[:], id_sb[:])

            setup_ctx = tc.tile_pool(name="setup", bufs=1)
            spool = setup_ctx.__enter__()
            # Wr natural [o_sub, oc, h] then transpose -> wrt [h_sub, hc, o] (f32r)
            wr_nat = spool.tile([P, OC, H], F32R)
            nc.gpsimd.dma_start(wr_nat[:], wr.rearrange("(oc p) h -> p oc h", p=P))
            wrt = cpool.tile([P, HC, H], F32R)
            for hc in range(HC):
                pst = tp_psum.tile([P, 512], F32R, tag="pst")
                for oc in range(OC):
                    nc.tensor.transpose(
                        pst[:, bass.ts(oc, P)], wr_nat[:, oc, bass.ts(hc, P)], id_r[:]
                    )
                nc.vector.tensor_copy(wrt[:, hc, :], pst[:])

            # Wq natural then transpose -> wqt [h_sub, hc, o] (f32, exact matmul)
            wq_nat = spool.tile([P, OC, H], F32)
            nc.sync.dma_start(wq_nat[:], wq.rearrange("(oc p) h -> p oc h", p=P))
            wqt = cpool.tile([P, HC, H], F32)
            for hc in range(HC):
                pst = tp_psum.tile([P, 512], F32, tag="pst")
                for oc in range(OC):
                    nc.tensor.transpose(
                        pst[:, bass.ts(oc, P)], wq_nat[:, oc, bass.ts(hc, P)], id_sb[:]
                    )
                nc.vector.tensor_copy(wqt[:, hc, :], pst[:])

            # bias vectors as [o_sub, oc]
            bq_sb = cpool.tile([P, OC], F32)
            nc.sync.dma_start(bq_sb[:], bq.rearrange("one (oc p) -> p (one oc)", p=P))
            br_sb = cpool.tile([P, OC], F32)
            nc.sync.dma_start(br_sb[:], br.rearrange("one (oc p) -> p (one oc)", p=P))
            bqbr = cpool.tile([P, OC], F32)
            nc.vector.tensor_add(bqbr[:], bq_sb[:], br_sb[:])
            v_f32 = cpool.tile([P, OC], F32)
            nc.sync.dma_start(v_f32[:], v.rearrange("one (oc p) -> p (one oc)", p=P))
            v_sb = cpool.tile([P, OC], F32R)
            nc.vector.tensor_copy(v_sb[:], v_f32[:])

            # ---- q = query @ Wq.T + bq (+ br), laid out [o_sub, oc, n] ----
            q_nat = spool.tile([NLOC, H], F32)
            nc.sync.dma_start(q_nat[:], query[:])
            qt = cpool.tile([P, HC, NLOC], F32)
            pstq = tp_psum.tile([P, 512], F32, tag="pst")
            for hc in range(HC):
                nc.tensor.transpose(
                    pstq[:, hc * NLOC : (hc + 1) * NLOC],
                    q_nat[:, bass.ts(hc, P)],
                    id_sb[:NLOC, :NLOC],
                )
            nc.vector.tensor_copy(qt[:], pstq[:, : HC * NLOC])
            qb = cpool.tile([P, OC, NLOC], F32)
            for oc in range(OC):
                psq = mm_psum.tile([P, 512], F32, tag="mm")
                for hc in range(HC):
                    nc.tensor.matmul(
                        psq[:, :NLOC],
                        wqt[:, hc, bass.ts(oc, P)],
                        qt[:, hc, :],
                        start=(hc == 0),
                        stop=(hc == HC - 1),
                    )
                nc.scalar.activation(
                    qb[:, oc, :], psq[:, :NLOC], IDENT, bias=bqbr[:, oc : oc + 1]
                )

            setup_ctx.__exit__(None, None, None)

            # ---- main loop over (n, lc) ----
            for n in range(NLOC):
                logits_n = log_pool.tile([1, L], F32, tag="logits_n")
                slabs = [
                    slab_pool.tile([P, L], F32, tag=f"slab{oc}", name=f"slab{oc}")
                    for oc in range(OC)
                ]
                for lc in range(LC):
                    # natural ref tile [l_sub(p), ls, h]
                    rn = rn_pool.tile([P, LS, H], F32R, tag="rn")
                    nc.gpsimd.dma_start(
                        rn[:],
                        ref[n].rearrange(
                            "(lc ls p) h -> lc p ls h", lc=LC, ls=LS, p=P
                        )[lc],
                    )
                    # transpose to [h_sub, hc, l] f32r: 4 transposes into one
                    # psum bank, then one wide rounding copy
                    rt = rt_pool.tile([P, HC, 512], F32R, tag="rt")
                    for hc in range(HC):
                        pst = tp_psum.tile([P, 512], F32R, tag="pst")
                        for ls in range(LS):
                            nc.tensor.transpose(
                                pst[:, bass.ts(ls, P)], rn[:, ls, bass.ts(hc, P)], id_r[:]
                            )
                        nc.vector.tensor_copy(rt[:, hc, :], pst[:])

                    pslog = log_psum.tile([1, 512], F32, tag="pslog")
                    for oc in range(OC):
                        psr = mm_psum.tile([P, 512], F32, tag="mm")
                        for hc in range(HC):
                            nc.tensor.matmul(
                                psr[:],
                                wrt[:, hc, bass.ts(oc, P)],
                                rt[:, hc, :],
                                start=(hc == 0),
                                stop=(hc == HC - 1),
                            )
                        # r slab slice: psum + br
                        if oc % 2 == 0:
                            nc.vector.tensor_scalar_add(
                                slabs[oc][:, bass.ts(lc, 512)], psr[:], br_sb[:, oc : oc + 1]
                            )
                        else:
                            nc.scalar.activation(
                                slabs[oc][:, bass.ts(lc, 512)], psr[:], IDENT,
                                bias=br_sb[:, oc : oc + 1],
                            )
                        # tanh(q + r) -> f32r
                        th = out_pool.tile([P, 512], F32R, tag="th")
                        nc.scalar.activation(
                            th[:], psr[:], TANH, bias=qb[:, oc, n : n + 1]
                        )
                        # logits partial: v . tanh, contract over o_sub
                        nc.tensor.matmul(
                            pslog[:],
                            v_sb[:, oc : oc + 1],
                            th[:],
                            start=(oc == 0),
                            stop=(oc == OC - 1),
                        )
                    # logits tail: 10 * tanh(acc)
                    lg = log_pool.tile([1, 512], F32, tag="lg")
                    nc.scalar.activation(lg[:], pslog[:], TANH)
                    nc.vector.tensor_scalar_mul(
                        logits_n[:, bass.ts(lc, 512)], lg[:], CLIPPING
                    )
                for oc in range(OC):
                    nc.sync.dma_start(r_out[n, bass.ts(oc, P), :], slabs[oc][:])
                nc.sync.dma_start(logits_out[n : n + 1, :], logits_n[:])

    _split_sync_waits(nc, max_waits=1)
    return nc


_NC_CACHE = None


def _get_nc():
    global _NC_CACHE
    if _NC_CACHE is None:
        _NC_CACHE = build_kernel()
    return _NC_CACHE


def kernel(query, ref, Wq, bq, Wr, br, v):
    from concourse.bass_utils import run_bass_kernel_spmd

    import ml_dtypes

    query = np.ascontiguousarray(query, dtype=np.float32)
    ref = np.ascontiguousarray(ref, dtype=np.float32)
    ident = np.eye(P, dtype=np.float32)
    ident_bf = np.eye(P, dtype=ml_dtypes.bfloat16)
    shared = {
        "wq": np.ascontiguousarray(Wq, dtype=np.float32),
        "bq": np.ascontiguousarray(bq, dtype=np.float32).reshape(1, H),
        "wr": np.ascontiguousarray(Wr, dtype=np.float32),
        "br": np.ascontiguousarray(br, dtype=np.float32).reshape(1, H),
        "v": np.ascontiguousarray(v, dtype=np.float32).reshape(1, H),
        "ident": ident,
    }
    in_maps = []
    for c in range(NCORES):
        sl = slice(c * NLOC, (c + 1) * NLOC)
        in_maps.append({"query": query[sl], "ref": ref[sl], **shared})

    nc = _get_nc()
    res = run_bass_kernel_spmd(nc, in_maps, core_ids=list(range(NCORES)))
    r_full = np.concatenate([res.results[c]["r_out"] for c in range(NCORES)], axis=0)
    logits_full = np.concatenate(
        [res.results[c]["logits_out"] for c in range(NCORES)], axis=0
    )
    return (r_full, logits_full)


# revision 13
# speedup vs baseline: 1.3229x; 1.1090x over previous
"""Trainium2 Bass kernel for the additive-attention module:
    q = query @ Wq.T + bq                                  (N, H)
    r = einsum("nlh,oh->nol", ref, Wr) + br[None,:,None]   (N, O, L)
    logits = 10 * tanh(einsum("h,nhl->nl", v, tanh(q[:,:,None] + r)))
Returns (r, logits).

Data-parallel over batch N across 8 NeuronCores (16 rows each); params
replicated. Per core: PE transposes ref tiles to put H on partitions,
fp32r matmuls against the pre-transposed Wr, ACT applies tanh with the
per-(n,o) bias, a second PE reduction dots with v.
"""
import sys

sys.path.insert(0, "/opt/trn_rl_repo")

import numpy as np
import concourse.bass as bass
import concourse.tile as tile
from concourse import mybir
from concourse.vector_clock import ScopedClock

F32 = mybir.dt.float32
F32R = mybir.dt.float32r
TANH = mybir.ActivationFunctionType.Tanh
IDENT = mybir.ActivationFunctionType.Identity

N, L, H = 128, 2048, 512
CLIPPING = 10.0
NCORES = 8
NLOC = N // NCORES          # 16 batch rows per core
P = 128                     # partitions
HC = H // P                 # 4 contraction chunks
OC = H // P                 # 4 output-row chunks
LC = L // 512               # 4 l-chunks of 512
LS = 512 // P               # 4 l-subtiles per chunk

_ENGINE_HANDLES = {
    mybir.EngineType.PE: "tensor",
    mybir.EngineType.DVE: "vector",
    mybir.EngineType.Activation: "scalar",
    mybir.EngineType.Pool: "gpsimd",
    mybir.EngineType.SP: "sync",
}


def _split_sync_waits(nc, max_waits=1):
    """This walrus build rejects instructions carrying more than a couple of
    semaphore waits; move the excess onto same-engine NOPs placed right
    before the instruction (same program-order semantics)."""
    for bb in nc.main_func.blocks:
        new_insts = []
        for inst in bb.instructions:
            si = inst.sync_info
            if si is not None and si.on_wait is not None and len(si.on_wait) > max_waits:
                waits = list(si.on_wait)
                excess = waits[: len(waits) - max_waits]
                handle = getattr(nc, _ENGINE_HANDLES[inst.engine])
                for j in range(0, len(excess), max_waits):
                    nop = handle.nop(nofuse=True).ins
                    for b2 in nc.main_func.blocks:
                        if b2.instructions and b2.instructions[-1] is nop:
                            b2.instructions.pop()
                            break
                    nsi = nop.sync_info or mybir.SyncInfo(on_wait=[], on_update=[])
                    nsi.on_wait = excess[j : j + max_waits]
                    nop.sync_info = nsi
                    new_insts.append(nop)
                si.on_wait = waits[len(waits) - max_waits :]
                inst.sync_info = si
            new_insts.append(inst)
        bb.instructions = new_insts


class _TC(tile.TileContext):
    """TileContext whose tail drain splits its sem waits the same way."""

    MAX_WAITS = 1

    def _drain_and_barrier(self, tick_clock, wait_clock):
        drain_inst = self.nc.sync.drain()
        wait_clock.add_sem_waits(
            drain_inst.ins, ScopedClock({None: tick_clock.global_clock})
        )
        waits = list(drain_inst.ins.sync_info.on_wait)
        if len(waits) > self.MAX_WAITS:
            si = drain_inst.ins.sync_info
            si.on_wait = waits[: self.MAX_WAITS]
            drain_inst.ins.sync_info = si
            for i in range(self.MAX_WAITS, len(waits), self.MAX_WAITS):
                extra = self.nc.sync.drain()
                esi = extra.ins.sync_info or mybir.SyncInfo(on_wait=[], on_update=[])
                esi.on_wait = waits[i : i + self.MAX_WAITS]
                extra.ins.sync_info = esi
        self.nc.all_engine_barrier()
        assert self.sems is not None
        popped = self.nc._tile_sem_poison_stack.pop()
        assert popped is self._sem_poison
        self.nc.clear_and_free_semaphores(list(self.sems.allocated().values()))
        self.nc.all_engine_barrier()


def build_kernel():
    nc = bass.Bass(trn_type="TRN2")

    query = nc.dram_tensor("query", [NLOC, H], F32, kind="ExternalInput")
    ref = nc.dram_tensor("ref", [NLOC, L, H], F32, kind="ExternalInput")
    wq = nc.dram_tensor("wq", [H, H], F32, kind="ExternalInput")
    bq = nc.dram_tensor("bq", [1, H], F32, kind="ExternalInput")
    wr = nc.dram_tensor("wr", [H, H], F32, kind="ExternalInput")
    br = nc.dram_tensor("br", [1, H], F32, kind="ExternalInput")
    v = nc.dram_tensor("v", [1, H], F32, kind="ExternalInput")
    ident = nc.dram_tensor("ident", [P, P], F32, kind="ExternalInput")

    r_out = nc.dram_tensor("r_out", [NLOC, H, L], F32, kind="ExternalOutput")
    logits_out = nc.dram_tensor("logits_out", [NLOC, L], F32, kind="ExternalOutput")

    with _TC(nc) as tc:
        with (
            tc.tile_pool(name="const", bufs=1) as cpool,
            tc.tile_pool(name="ref_nat", bufs=4) as rn_pool,
            tc.tile_pool(name="ref_t", bufs=3) as rt_pool,
            tc.tile_pool(name="outs", bufs=6) as out_pool,
            tc.tile_pool(name="slabs", bufs=2) as slab_pool,
            tc.tile_pool(name="logits_sb", bufs=2) as log_pool,
            tc.tile_pool(name="mm_psum", bufs=3, space="PSUM") as mm_psum,
            tc.tile_pool(name="tp_psum", bufs=3, space="PSUM") as tp_psum,
            tc.tile_pool(name="log_psum", bufs=2, space="PSUM") as log_psum,
        ):
            # ---- constants / weights ----
            id_sb = cpool.tile([P, P], F32)
            nc.sync.dma_start(id_sb[:], ident[:])
            id_r = cpool.tile([P, P], F32R)
            nc.vector.tensor_copy(id_r[:], id_sb[:])

            setup_ctx = tc.tile_pool(name="setup", bufs=1)
            spool = setup_ctx.__enter__()
            # Wr natural [o_sub, oc, h] then transpose -> wrt [h_sub, hc, o] (f32r)
            wr_nat = spool.tile([P, OC, H], F32R)
            nc.gpsimd.dma_start(wr_nat[:], wr.rearrange("(oc p) h -> p oc h", p=P))
            wrt = cpool.tile([P, HC, H], F32R)
            for hc in range(HC):
                pst = tp_psum.tile([P, 512], F32R, tag="pst")
                for oc in range(OC):
                    nc.tensor.transpose(
                        pst[:, bass.ts(oc, P)], wr_nat[:, oc, bass.ts(hc, P)], id_r[:]
                    )
                nc.vector.tensor_copy(wrt[:, hc, :], pst[:])

            # Wq natural then transpose -> wqt [h_sub, hc, o] (f32, exact matmul)
            wq_nat = spool.tile([P, OC, H], F32)
            nc.sync.dma_start(wq_nat[:], wq.rearrange("(oc p) h -> p oc h", p=P))
            wqt = cpool.tile([P, HC, H], F32)
            for hc in range(HC):
                pst = tp_psum.tile([P, 512], F32, tag="pst")
                for oc in range(OC):
                    nc.tensor.transpose(
                        pst[:, bass.ts(oc, P)], wq_nat[:, oc, bass.ts(hc, P)], id_sb[:]
                    )
                nc.vector.tensor_copy(wqt[:, hc, :], pst[:])

            # bias vectors as [o_sub, oc]
            bq_sb = cpool.tile([P, OC], F32)
            nc.sync.dma_start(bq_sb[:], bq.rearrange("one (oc p) -> p (one oc)", p=P))
            br_sb = cpool.tile([P, OC], F32)
            nc.sync.dma_start(br_sb[:], br.rearrange("one (oc p) -> p (one oc)", p=P))
            bqbr = cpool.tile([P, OC], F32)
            nc.vector.tensor_add(bqbr[:], bq_sb[:], br_sb[:])
            v_f32 = cpool.tile([P, OC], F32)
            nc.sync.dma_start(v_f32[:], v.rearrange("one (oc p) -> p (one oc)", p=P))
            v_sb = cpool.tile([P, OC], F32R)
            nc.vector.tensor_copy(v_sb[:], v_f32[:])

            # ---- q = query @ Wq.T + bq (+ br), laid out [o_sub, oc, n] ----
            q_nat = spool.tile([NLOC, H], F32)
            nc.sync.dma_start(q_nat[:], query[:])
            qt = cpool.tile([P, HC, NLOC], F32)
            pstq = tp_psum.tile([P, 512], F32, tag="pst")
            for hc in range(HC):
                nc.tensor.transpose(
                    pstq[:, hc * NLOC : (hc + 1) * NLOC],
                    q_nat[:, bass.ts(hc, P)],
                    id_sb[:NLOC, :NLOC],
                )
            nc.vector.tensor_copy(qt[:], pstq[:, : HC * NLOC])
            qb = cpool.tile([P, OC, NLOC], F32)
            for oc in range(OC):
                psq = mm_psum.tile([P, 512], F32, tag="mm")
                for hc in range(HC):
                    nc.tensor.matmul(
                        psq[:, :NLOC],
                        wqt[:, hc, bass.ts(oc, P)],
                        qt[:, hc, :],
                        start=(hc == 0),
                        stop=(hc == HC - 1),
                    )
                nc.scalar.activation(
                    qb[:, oc, :], psq[:, :NLOC], IDENT, bias=bqbr[:, oc : oc + 1]
                )

            setup_ctx.__exit__(None, None, None)

            # ---- main loop over (n, lc) ----
            for n in range(NLOC):
                logits_n = log_pool.tile([1, L], F32, tag="logits_n")
                slabs = [
                    slab_pool.tile([P, L], F32, tag=f"slab{oc}", name=f"slab{oc}")
                    for oc in range(OC)
                ]
                for lc in range(LC):
                    # natural ref tile [l_sub(p), ls, h]
                    rn = rn_pool.tile([P, LS, H], F32R, tag="rn")
                    nc.gpsimd.dma_start(
                        rn[:],
                        ref[n].rearrange(
                            "(lc ls p) h -> lc p ls h", lc=LC, ls=LS, p=P
                        )[lc],
                    )
                    # transpose to [h_sub, hc, l] f32r: 4 transposes into one
                    # psum bank, then one wide rounding copy
                    rt = rt_pool.tile([P, HC, 512], F32R, tag="rt")
                    for hc in range(HC):
                        pst = tp_psum.tile([P, 512], F32R, tag="pst")
                        for ls in range(LS):
                            nc.tensor.transpose(
                                pst[:, bass.ts(ls, P)], rn[:, ls, bass.ts(hc, P)], id_r[:]
                            )
                        nc.vector.tensor_copy(rt[:, hc, :], pst[:])

                    pslog = log_psum.tile([1, 512], F32, tag="pslog")
                    for oc in range(OC):
                        psr = mm_psum.tile([P, 512], F32, tag="mm")
                        for hc in range(HC):
                            nc.tensor.matmul(
                                psr[:],
                                wrt[:, hc, bass.ts(oc, P)],
                                rt[:, hc, :],
                                start=(hc == 0),
                                stop=(hc == HC - 1),
                            )
                        # r slab slice: psum + br
                        if oc % 2 == 0:
                            nc.vector.tensor_scalar_add(
                                slabs[oc][:, bass.ts(lc, 512)], psr[:], br_sb[:, oc : oc + 1]
                            )
                        else:
                            nc.scalar.activation(
                                slabs[oc][:, bass.ts(lc, 512)], psr[:], IDENT,
                                bias=br_sb[:, oc : oc + 1],
                            )
                        # tanh(q + r) -> f32r
                        th = out_pool.tile([P, 512], F32R, tag="th")
                        nc.scalar.activation(
                            th[:], psr[:], TANH, bias=qb[:, oc, n : n + 1]
                        )
                        # logits partial: v . tanh, contract over o_sub
                        nc.tensor.matmul(
                            pslog[:],
                            v_sb[:, oc : oc + 1],
                            th[:],
                            start=(oc == 0),
                            stop=(oc == OC - 1),
                        )
                    # logits tail: 10 * tanh(acc)
                    lg = log_pool.tile([1, 512], F32, tag="lg")
                    nc.scalar.activation(lg[:], pslog[:], TANH)
                    nc.vector.tensor_scalar_mul(
                        logits_n[:, bass.ts(lc, 512)], lg[:], CLIPPING
                    )
                    if lc == 1:
                        for oc in range(OC):
                            nc.sync.dma_start(
                                r_out[n, bass.ts(oc, P), 0:1024], slabs[oc][:, 0:1024]
                            )
                    elif lc == 3:
                        for oc in range(OC):
                            nc.sync.dma_start(
                                r_out[n, bass.ts(oc, P), 1024:2048],
                                slabs[oc][:, 1024:2048],
                            )
                nc.sync.dma_start(logits_out[n : n + 1, :], logits_n[:])

    _split_sync_waits(nc, max_waits=1)
    return nc


_NC_CACHE = None


def _get_nc():
    global _NC_CACHE
    if _NC_CACHE is None:
        _NC_CACHE = build_kernel()
    return _NC_CACHE


def kernel(query, ref, Wq, bq, Wr, br, v):
    from concourse.bass_utils import run_bass_kernel_spmd

    import ml_dtypes

    query = np.ascontiguousarray(query, dtype=np.float32)
    ref = np.ascontiguousarray(ref, dtype=np.float32)
    ident = np.eye(P, dtype=np.float32)
    ident_bf = np.eye(P, dtype=ml_dtypes.bfloat16)
    shared = {
        "wq": np.ascontiguousarray(Wq, dtype=np.float32),
        "bq": np.ascontiguousarray(bq, dtype=np.float32).reshape(1, H),
        "wr": np.ascontiguousarray(Wr, dtype=np.float32),
        "br": np.ascontiguousarray(br, dtype=np.float32).reshape(1, H),
        "v": np.ascontiguousarray(v, dtype=np.float32).reshape(1, H),
        "ident": ident,
    }
    in_maps = []
    for c in range(NCORES):
        sl = slice(c * NLOC, (c + 1) * NLOC)
        in_maps.append({"query": query[sl], "ref": ref[sl], **shared})

    nc = _get_nc()
    res = run_bass_kernel_spmd(nc, in_maps, core_ids=list(range(NCORES)))
    r_full = np.concatenate([res.results[c]["r_out"] for c in range(NCORES)], axis=0)
    logits_full = np.concatenate(
        [res.results[c]["logits_out"] for c in range(NCORES)], axis=0
    )
    return (r_full, logits_full)
